# revision 63
# baseline (speedup 1.0000x reference)
"""Trainium2 Bass kernel for the 3-layer dual-head GAT (nn_DualHeadGAT), v2.

Strategy (dst-range sharded edges, bf16 tables, no layer-0 AllGather):
  - Nodes split contiguously across 8 cores (6250 each); edges sorted by
    destination so all segment reductions are core-local.
  - Per-node feature tables are bf16 rows [h | es | ed | pad]:
    layers 0/1: 384 cols (768B rows), layer 2: 128 cols (256B rows).
  - Layer 0's table is computed IN FULL on every core from x (x is tiny),
    laid out *rotated* so each core's own nodes come first; this removes
    the layer-0 AllGather. Layers 1/2 AllGather bf16 tables.
  - Edges are processed in superblocks of SB dst-blocks: one dma_gather
    per (lo, hi, ed) region per superblock (Pool-engine SWDGE call
    overhead is ~1us each), and all per-edge elementwise math runs as one
    instruction per superblock.  Per edge, the src row (h+es) is gathered
    by rotated/global src id (int16 split at 32768); the dst row slice
    (ed) is gathered from the core-local table with local dst ids.
  - Scatter-add per 128-dst-node block via one-hot matmul on the PE:
      U = sum_j M_j^T @ [ea*h | ea],  M[e, n] = (dst_local[e] == n)
    with M and rhs in bf16 (PSUM accumulates f32). Self-loops are folded
    in as one extra identity-matrix slot per block.
  - Per-node normalize: out = relu(U[:, :OC]/U[:, OC:] + b); next-layer
    rows h2 = out @ W' via PE transpose + f32r matmul (full precision
    weights), written bf16.

Self-contained: hardcodes the problem shapes; host-side preprocessing of
edge_index is pure index manipulation. All float math runs on device.
"""
import math
from contextlib import ExitStack
import numpy as np
import ml_dtypes

import concourse.bass as bass
import concourse.bacc as bacc
import concourse.mybir as mybir
import concourse.tile as tile
from concourse.bass_utils import run_bass_kernel_spmd
from concourse.tile_rust import add_dep_helper

F32 = mybir.dt.float32
F32R = mybir.dt.float32r
BF16 = mybir.dt.bfloat16
I16 = mybir.dt.int16
BF_NP = ml_dtypes.bfloat16

P = 128
NEG = 0.2


class Geo:
    def __init__(self, n=50000, ncores=8, split=32768):
        self.N = n
        self.NCORES = ncores
        self.NPD = n // ncores
        self.NBLK = math.ceil(self.NPD / P)
        self.NBLK_ALL = math.ceil(n / P)
        self.SPLIT = split


GEO = Geo()

LAYERS = [(2, 4, 64), (256, 4, 64), (256, 1, 2)]
ROWW = [384, 384, 128]     # bf16 row width of each layer's table
USED = [264, 264, 4]       # h | es | ed used cols
ES0 = [256, 256, 2]        # es col offset within row

SB = 2                     # dst-blocks per gather superblock
CALL_SLOTS = 8            # max 128-edge slots per dma_gather call
SCRATCH = 16384           # SWDGE ring bytes (ring descs = SCRATCH//16)
SINGLE_PACKET = True      # multi-packet SWDGE for big calls
QROT = 4                   # rotate gather calls over this many SWDGE queues
                           # (sim's sem-lane/queue affinity check requires 1)
STAGE = 0                  # 0=full, 1=phaseA only, 2=+L0, 3=+AG1+L1


# --------------------------------------------------------------------------
# host preprocessing
# --------------------------------------------------------------------------

def _wrap16(seq):
    """dma_gather index layout: idx i lives at [i % 16, i // 16]; tiled to
    128 partitions."""
    seq = np.asarray(seq, np.int16)
    a = seq.reshape(-1, 16).T
    return np.tile(a, (8, 1))


def _build_struct(src_key, dst_all, geo, sbsz):
    """Slot structure for one gather-key function, superblock-ordered.

    Slot layout: per superblock of sbsz dst-blocks:
    [lo(k0) lo(k1) ... hi(k0) hi(k1) ...]."""
    N, NCORES, NPD, NBLK, SPLIT = (geo.N, geo.NCORES, geo.NPD, geo.NBLK,
                                   geo.SPLIT)
    cnt_lo = np.zeros((NCORES, NBLK), np.int64)
    cnt_hi = np.zeros((NCORES, NBLK), np.int64)
    parts = {}
    for d in range(NCORES):
        for k in range(NBLK):
            i = d * NBLK + k
            keys = src_key(d, i)
            dsts = dst_all[i]
            lo = keys < SPLIT
            parts[(d, k)] = (keys[lo], dsts[lo], keys[~lo] - SPLIT, dsts[~lo])
            cnt_lo[d, k] = int(lo.sum())
            cnt_hi[d, k] = int((~lo).sum())

    S_lo = np.ceil(cnt_lo.max(axis=0) / P).astype(np.int64)
    S_hi = np.ceil(cnt_hi.max(axis=0) / P).astype(np.int64)
    olo = np.concatenate([[0], np.cumsum(S_lo)]).astype(np.int64)
    ohi = np.concatenate([[0], np.cumsum(S_hi)]).astype(np.int64)
    T_lo, T_hi = int(olo[-1]), int(ohi[-1])

    blk_lo_pos = np.zeros(NBLK, np.int64)
    blk_hi_pos = np.zeros(NBLK, np.int64)
    sb_bounds = []
    pos = 0
    for k0 in range(0, NBLK, sbsz):
        k1 = min(k0 + sbsz, NBLK)
        Llo = int(S_lo[k0:k1].sum())
        Lhi = int(S_hi[k0:k1].sum())
        pp = pos
        for k in range(k0, k1):
            blk_lo_pos[k] = pp
            pp += S_lo[k]
        for k in range(k0, k1):
            blk_hi_pos[k] = pp
            pp += S_hi[k]
        sb_bounds.append((k0, k1, pos, Llo, Lhi))
        pos += Llo + Lhi
    T = pos

    tabs = []
    for d in range(NCORES):
        idx_lo = np.zeros((P, 8 * max(T_lo, 1)), np.int16)
        idx_hi = np.zeros((P, 8 * max(T_hi, 1)), np.int16)
        idx_ed = np.zeros((P, 8 * max(T, 1)), np.int16)
        dloc = np.full((P, max(T, 1)), 999.0, BF_NP)

        def place(pos0, nslots, keys, dsts, base, tab):
            if nslots == 0:
                return
            buf = np.zeros(nslots * P, np.int16)
            buf[:len(keys)] = keys.astype(np.int16)
            tab[:, 8 * pos0:8 * (pos0 + nslots)] = _wrap16(buf)

        for k in range(NBLK):
            kl, dl, kh, dh = parts[(d, k)]
            slo, shi = int(S_lo[k]), int(S_hi[k])
            base = d * NPD + k * P
            place(int(olo[k]), slo, kl, dl, base, idx_lo)
            place(int(ohi[k]), shi, kh, dh, base, idx_hi)

            for pos0, nsl, dd in ((int(blk_lo_pos[k]), slo, dl),
                                  (int(blk_hi_pos[k]), shi, dh)):
                if nsl == 0:
                    continue
                buf = np.zeros(nsl * P, np.int16)
                buf[:len(dd)] = (dd - d * NPD).astype(np.int16)
                idx_ed[:, 8 * pos0:8 * (pos0 + nsl)] = _wrap16(buf)
                loc = np.full(nsl * P, 999.0, np.float32)
                loc[:len(dd)] = (dd - base).astype(np.float32)
                dloc[:, pos0:pos0 + nsl] = \
                    loc.reshape(nsl, P).T.astype(BF_NP)
        tabs.append((idx_lo, idx_hi, idx_ed, dloc))

    plan = {
        "S_lo": tuple(int(s) for s in S_lo),
        "S_hi": tuple(int(s) for s in S_hi),
        "olo": tuple(int(o) for o in olo),
        "ohi": tuple(int(o) for o in ohi),
        "blk_lo": tuple(int(v) for v in blk_lo_pos),
        "blk_hi": tuple(int(v) for v in blk_hi_pos),
        "sb": tuple(sb_bounds),
        "T": T, "T_lo": T_lo, "T_hi": T_hi,
    }
    return plan, tabs


def _host_prep(x, edge_index, weights, geo=GEO):
    N, NCORES, NPD, NBLK = geo.N, geo.NCORES, geo.NPD, geo.NBLK
    src = np.asarray(edge_index[0]).astype(np.int64)
    dst = np.asarray(edge_index[1]).astype(np.int64)
    perm = np.argsort(dst, kind="stable")
    s_src = src[perm].astype(np.int64)
    s_dst = dst[perm].astype(np.int64)

    starts, stops = [], []
    for d in range(NCORES):
        for k in range(NBLK):
            starts.append(d * NPD + k * P)
            stops.append(min(d * NPD + (k + 1) * P, (d + 1) * NPD))
    e_lo = np.searchsorted(s_dst, starts)
    e_hi = np.searchsorted(s_dst, stops)
    src_blk = {i: s_src[e_lo[i]:e_hi[i]] for i in range(NCORES * NBLK)}
    dst_blk = {i: s_dst[e_lo[i]:e_hi[i]] for i in range(NCORES * NBLK)}

    # structure A: layer-0 rotated keys; structure B: global keys
    planA, tabsA = _build_struct(
        lambda d, i: (src_blk[i] - d * NPD) % N, dst_blk, geo, SB)
    planB, tabsB = _build_struct(
        lambda d, i: src_blk[i], dst_blk, geo, SB)

    x = np.asarray(x, np.float32)
    bzero = tuple(bool(np.all(np.asarray(w[3]) == 0)) for w in weights)
    in_maps = []
    for d in range(NCORES):
        ilA, ihA, ieA, dlA = tabsA[d]
        ilB, ihB, ieB, dlB = tabsB[d]
        rot = np.roll(np.arange(N), -d * NPD)
        m = {
            "xTr": np.ascontiguousarray(x[rot].T),
            "idx_loA": ilA, "idx_hiA": ihA, "idx_edA": ieA, "dlocA": dlA,
            "idx_loB": ilB, "idx_hiB": ihB, "idx_edB": ieB, "dlocB": dlB,
            "iota_bf": np.tile(np.arange(P, dtype=BF_NP)[None, :], (P, 1)),
            "ident_bf": np.eye(P, dtype=BF_NP),
            "ident_f": np.eye(P, dtype=np.float32),
        }
        for li, (W, a_s, a_d, b) in enumerate(weights):
            fin, H, O = LAYERS[li]
            W = np.asarray(W, np.float32)
            a_s = np.asarray(a_s, np.float32)
            a_d = np.asarray(a_d, np.float32)
            b = np.asarray(b, np.float32)
            As = np.zeros((H * O, H), np.float32)
            Ad = np.zeros((H * O, H), np.float32)
            for h in range(H):
                As[h * O:(h + 1) * O, h] = a_s[h]
                Ad[h * O:(h + 1) * O, h] = a_d[h]
            m[f"W{li}"] = W
            m[f"WT{li}"] = np.ascontiguousarray(W.T)
            m[f"As{li}"] = As
            m[f"Ad{li}"] = Ad
            m[f"b{li}"] = np.tile(b[None, :], (P, 1))
        in_maps.append(m)

    return in_maps, {"A": planA, "B": planB, "bzero": bzero}


# --------------------------------------------------------------------------
# device program
# --------------------------------------------------------------------------

def build_program(plans, geo=GEO, repeat=1):
    N, NCORES, NPD = geo.N, geo.NCORES, geo.NPD
    nc = bacc.Bacc("TRN2", target_bir_lowering=False, debug=False,
                   num_devices=NCORES, num_swdge_queues=4,
                   dynamic_dma_scratch_size=SCRATCH)

    t_in = {}

    def inp(name, shape, dt=F32):
        t_in[name] = nc.dram_tensor(name, shape, dt, kind="ExternalInput").ap()

    pA, pB = plans["A"], plans["B"]
    inp("xTr", [2, N])
    inp("idx_loA", [P, 8 * max(pA["T_lo"], 1)], I16)
    inp("idx_hiA", [P, 8 * max(pA["T_hi"], 1)], I16)
    inp("idx_edA", [P, 8 * max(pA["T"], 1)], I16)
    inp("dlocA", [P, max(pA["T"], 1)], BF16)
    inp("idx_loB", [P, 8 * max(pB["T_lo"], 1)], I16)
    inp("idx_hiB", [P, 8 * max(pB["T_hi"], 1)], I16)
    inp("idx_edB", [P, 8 * max(pB["T"], 1)], I16)
    inp("dlocB", [P, max(pB["T"], 1)], BF16)
    inp("iota_bf", [P, P], BF16)
    inp("ident_bf", [P, P], BF16)
    inp("ident_f", [P, P])
    for li, (fin, H, O) in enumerate(LAYERS):
        OC = H * O
        inp(f"W{li}", [fin, OC])
        inp(f"WT{li}", [OC, fin])
        inp(f"As{li}", [OC, H])
        inp(f"Ad{li}", [OC, H])
        inp(f"b{li}", [P, OC])

    out_own = nc.dram_tensor("out", [NPD, 2], F32, kind="ExternalOutput").ap()

    tbl = [nc.dram_tensor("tbl0", [N, ROWW[0]], BF16, kind="Internal").ap()]
    for li in (1, 2):
        tbl.append(nc.dram_tensor(f"tbl{li}", [N, ROWW[li]], BF16,
                                  kind="Internal", addr_space="Shared").ap())
    howns = [None,
             nc.dram_tensor("hown1", [NPD, ROWW[1]], BF16, kind="Internal").ap(),
             nc.dram_tensor("hown2", [NPD, ROWW[2]], BF16, kind="Internal").ap()]
    aux = {
        "hown2p": nc.dram_tensor("hown2p", [NPD, 8], BF16,
                                 kind="Internal").ap(),
        "tbl2p": nc.dram_tensor("tbl2p", [N, 8], BF16, kind="Internal",
                                addr_space="Shared").ap(),
    }

    with tile.TileContext(nc) as tc:
        _emit(tc, t_in, out_own, tbl, howns, aux, plans, geo, repeat)

    # Post-scheduling: pin each gather's SWDGE queue to its assigned DMASW
    # sem lane (lane rotates per Pool-DMA inst in scheduled order; the sem
    # of lane L must always be updated from one queue, so queue = L % QROT).
    if QROT > 1:
        for blk in nc.m.functions[0].blocks:
            for inst in blk.instructions:
                if isinstance(inst, mybir.InstDMAGatherAnt):
                    proc = getattr(inst, "bass_scheduled_proc", None)
                    if proc is not None and 11 <= proc <= 18:
                        inst.queue_num = (proc - 11) % QROT
    nc.compile()
    return nc


def _emit(tc, t_in, out_own, tbl, howns, aux, plans, geo, repeat):
    nc = tc.nc
    pA, pB = plans["A"], plans["B"]
    Lmax = max(lo + hi for pl in (pA, pB) for (_, _, _, lo, hi) in pl["sb"])

    ctx = ExitStack()
    sb_c = ctx.enter_context(tc.tile_pool(name="const", bufs=1))
    sb_i = ctx.enter_context(tc.tile_pool(name="idx", bufs=1))
    sb = ctx.enter_context(tc.tile_pool(name="work", bufs=2))
    sbg = ctx.enter_context(tc.tile_pool(name="gath", bufs=2))
    sbs = ctx.enter_context(tc.tile_pool(name="stage", bufs=2))
    ps = ctx.enter_context(tc.tile_pool(name="psum", bufs=2, space="PSUM"))
    ps_u = ctx.enter_context(tc.tile_pool(name="psum_u", bufs=2, space="PSUM"))

    # ---- persistent constants ----
    def load_const(name, shape, dt=F32):
        t = sb_c.tile(shape, dt, tag=name)
        nc.sync.dma_start(out=t[:], in_=t_in[name][:])
        return t

    c_iota = load_const("iota_bf", [P, P], BF16)
    c_identb = load_const("ident_bf", [P, P], BF16)
    c_identf = load_const("ident_f", [P, P])
    c_b = [load_const(f"b{li}", [P, LAYERS[li][1] * LAYERS[li][2]])
           for li in range(3)]

    # ---- per-structure gather index tiles (one structure resident) ----
    TLO = max(pA["T_lo"], pB["T_lo"], 1)
    THI = max(pA["T_hi"], pB["T_hi"], 1)
    TT = max(pA["T"], pB["T"], 1)

    def load_struct(sfx):
        p = pA if sfx == "A" else pB
        il = sb_i.tile([P, 8 * TLO], I16, tag="il")
        ih = sb_i.tile([P, 8 * THI], I16, tag="ih")
        ie = sb_i.tile([P, 8 * TT], I16, tag="ie")
        dl = sb_i.tile([P, TT], BF16, tag="dl")
        nc.sync.dma_start(out=il[:, 0:8 * max(p["T_lo"], 1)],
                          in_=t_in[f"idx_lo{sfx}"][:])
        nc.sync.dma_start(out=ih[:, 0:8 * max(p["T_hi"], 1)],
                          in_=t_in[f"idx_hi{sfx}"][:])
        nc.sync.dma_start(out=ie[:, 0:8 * max(p["T"], 1)],
                          in_=t_in[f"idx_ed{sfx}"][:])
        nc.sync.dma_start(out=dl[:, 0:max(p["T"], 1)],
                          in_=t_in[f"dloc{sfx}"][:])
        return il, ih, ie, dl

    # ---- W' = [W | W@As | W@Ad] per layer (f32, tiny) ----
    wprime = []
    for li, (fin, H, O) in enumerate(LAYERS):
        OC = H * O
        n_fin_t = math.ceil(fin / P)
        n_k_t = math.ceil(OC / P)
        kp = min(P, OC)
        tiles = []
        for fi in range(n_fin_t):
            fr = min(P, fin - fi * P)
            wp = sb_c.tile([P, OC + 2 * H], F32R, tag=f"wp{li}_{fi}")
            nc.sync.dma_start(out=wp[:fr, 0:OC],
                              in_=t_in[f"W{li}"][fi * P:fi * P + fr, :]
                              .bitcast(F32R))
            for ci, aname in ((0, f"As{li}"), (1, f"Ad{li}")):
                wa_fl = ps.tile([P, 264], F32, space="PSUM", tag="h0ps")
                wa_ps = wa_fl[:, 0:H]
                a_sb = sb.tile([P, n_k_t, H], F32, tag="a_in")
                nc.sync.dma_start(
                    out=a_sb[:kp, 0:n_k_t, :],
                    in_=t_in[aname][:].rearrange("(a p) h -> p a h", p=kp))
                wt_sb = sb.tile([P, n_k_t, P], F32, tag="wt_in")
                nc.sync.dma_start(
                    out=wt_sb[:kp, 0:n_k_t, 0:fr],
                    in_=t_in[f"WT{li}"][:, fi * P:fi * P + fr].rearrange(
                        "(a p) f -> p a f", p=kp))
                for ki in range(n_k_t):
                    kr = min(P, OC - ki * P)
                    nc.tensor.matmul(
                        out=wa_ps[:fr, :],
                        lhsT=wt_sb[:kr, ki, 0:fr],
                        rhs=a_sb[:kr, ki, :],
                        start=(ki == 0), stop=(ki == n_k_t - 1))
                nc.vector.tensor_copy(
                    out=wp[:fr, OC + ci * H:OC + (ci + 1) * H],
                    in_=wa_ps[:fr, :])
            tiles.append(wp)
        wprime.append(tiles)

    consts = (c_iota, c_identb, c_identf, c_b, wprime, load_struct)
    pools = (sb, sbg, sbs, ps, ps_u, sb_i)
    for rep in range(repeat):
        _emit_iter(tc, t_in, out_own, tbl, howns, aux, plans, geo, consts,
                   pools, Lmax)

    ctx.close()


def _emit_iter(tc, t_in, out_own, tbl, howns, aux, plans, geo, consts, pools,
               Lmax):
    nc = tc.nc
    N, NCORES, NPD, NBLK, NBLK_ALL, SPLIT = (geo.N, geo.NCORES, geo.NPD,
                                             geo.NBLK, geo.NBLK_ALL, geo.SPLIT)
    (c_iota, c_identb, c_identf, c_b, wprime, load_struct) = consts
    sb, sbg, sbs, ps, ps_u, sb_i = pools
    pA, pB = plans["A"], plans["B"]

    qctr = [0]
    TTmax = max(plans["A"]["T"], plans["B"]["T"], 1)

    def next_q():
        return 0

    # ================= phase A: full layer-0 table (rotated) =================
    XCHUNK = 384  # blocks of x columns per SBUF load
    tbl0_writes = []
    tbl0_local_writes = []  # writes covering table rows [0:NPD] (own nodes)

    def flush_tbl0(stage, blocks):
        full = [(j, gg) for (j, gg, nk) in blocks if nk == P]
        part = [(j, gg, nk) for (j, gg, nk) in blocks if nk != P]
        ws = []
        if full:
            j0, g0 = full[0]
            cnt = len(full)
            w = nc.sync.dma_start(
                out=tbl[0][g0 * P:(g0 + cnt) * P, 0:264].rearrange(
                    "(a p) c -> p a c", p=P),
                in_=stage[:, j0:j0 + cnt, :])
            ws.append(w)
        for (j, gg, nk) in part:
            w = nc.sync.dma_start(
                out=tbl[0][gg * P:gg * P + nk, 0:264],
                in_=stage[:nk, j, :])
            ws.append(w)
        return ws

    B_ST = 6
    g = 0
    eng_rot = [nc.vector, nc.scalar]   # Pool cannot read PSUM
    while g < NBLK_ALL:
        c0 = g * P
        ccols = min(XCHUNK, N - c0)
        nblk_c = math.ceil(ccols / P)
        xc = sb.tile([2, XCHUNK], F32R, tag="xc")
        nc.sync.dma_start(out=xc[:2, 0:ccols],
                          in_=t_in["xTr"][:, c0:c0 + ccols].bitcast(F32R))
        done = 0
        while done < nblk_c:
            grp = min(B_ST, nblk_c - done)
            stage = sbs.tile([P, B_ST, 264], BF16, tag="stA")
            blocks = []
            for j in range(grp):
                gb = g + done + j
                nk = min(P, N - gb * P)
                h0_ps = ps.tile([P, 264], F32, space="PSUM", tag="h0ps")
                nc.tensor.matmul(
                    out=h0_ps[:nk, :],
                    lhsT=xc[:2, (done + j) * P:(done + j) * P + nk],
                    rhs=wprime[0][0][:2, 0:264],
                    start=True, stop=True)
                eng = eng_rot[gb % 2]
                if eng is nc.scalar:
                    nc.scalar.copy(out=stage[:nk, j, :], in_=h0_ps[:nk, :])
                else:
                    eng.tensor_copy(out=stage[:nk, j, :], in_=h0_ps[:nk, :])
                blocks.append((j, gb, nk))
            ws = flush_tbl0(stage, blocks)
            tbl0_writes += ws
            if any(gg < NBLK for (_, gg, _) in blocks):
                tbl0_local_writes += ws
            done += grp
        g += nblk_c

    # barrier over just the own-node rows [0:NPD] — unblocks ed gathers and
    # self-row loads long before the full table is written
    bar_local = nc.gpsimd.engine_nop()
    for w in tbl0_local_writes:
        add_dep_helper(bar_local.ins, w.ins, reason="tbl0 local barrier")

    if STAGE == 1:
        return

    # ================= layers =================
    h_writes = []
    hw_byblock = {}

    n_layers = {0: 3, 2: 1, 3: 2}[STAGE]
    for li in range(n_layers):
        fin, H, O = LAYERS[li]
        OC = H * O
        RC = OC + H
        roww = ROWW[li]
        es0 = ES0[li]
        # ed col offset inside the gathered 128-col ed row: layers 0/1 gather
        # row cols [256:384] (es at 0, ed at H); layer 2 gathers [0:128]
        edo = H if li < 2 else 3
        last = (li == 2)
        pl = pA if li == 0 else pB
        if li == 0:
            struct = load_struct("A")
        elif li == 1:
            struct = load_struct("B")
        c_il, c_ih, c_ie, c_dl = struct
        S_lo, S_hi = pl["S_lo"], pl["S_hi"]
        olo, ohi = pl["olo"], pl["ohi"]
        blk_lo, blk_hi = pl["blk_lo"], pl["blk_hi"]

        prev_h_writes = h_writes
        prev_hw_byblock = hw_byblock
        if li == 0:
            ed_src = tbl[0][0:NPD, 256:384]
            self_src = tbl[0]
        elif li == 1:
            ed_src = howns[1][:, 256:384]
            self_src = howns[1]
        else:
            ed_src = howns[2][:, 0:128]
            self_src = howns[2]
        hw_byblock = {}
        h_writes = []

        cols_u = USED[li]
        nfull = NPD // P
        rem = NPD - nfull * P

        # funnel the ~13 h-write DMAs through one nop: consumers that need
        # "all h writes done" get a single wait slot instead of 13 — with
        # too many waits the sem-assignment coarsens to "wait on the AG",
        # which silently serializes phase E behind the collective.
        hbar = None
        if li >= 1:
            hbar = nc.vector.engine_nop()
            for w in prev_h_writes:
                add_dep_helper(hbar.ins, w.ins, reason="h-writes funnel")

        # ---- collectives first: Pool's in-order SEQ then holds the ed
        # gathers (emitted next) until the AG *dispatches* (= h-writes
        # done), so they stream inside the collective window instead of
        # congesting the previous layer's tail.
        if li == 1:
            ag = nc.gpsimd.collective_compute(
                "AllGather", mybir.AluOpType.bypass,
                replica_groups=[list(range(NCORES))],
                ins=[howns[1][:]], outs=[tbl[1][:]],
            )
            add_dep_helper(ag.ins, hbar.ins, reason="AG after h writes")
            src_dep = ag
        elif li == 2:
            # L2 rows use 4 of 128 cols — AllGather a packed [N, 8] table
            # (0.8MB vs 12.8MB) and locally expand into the 128-col-stride
            # gather table (src gathers need 256B-multiple row strides).
            pk = nc.sync.dma_start(out=aux["hown2p"][:],
                                   in_=howns[2][:, 0:8])
            add_dep_helper(pk.ins, hbar.ins, reason="pack after h writes")
            ag = nc.gpsimd.collective_compute(
                "AllGather", mybir.AluOpType.bypass,
                replica_groups=[list(range(NCORES))],
                ins=[aux["hown2p"][:]], outs=[aux["tbl2p"][:]],
            )
            add_dep_helper(ag.ins, pk.ins, reason="AG after pack")
            ex = nc.sync.dma_start(out=tbl[2][:, 0:8], in_=aux["tbl2p"][:])
            add_dep_helper(ex.ins, ag.ins, reason="expand after AG2")
            src_dep = ex

        # ---- phase E: ed gathers compacted into edc. For L1/L2 these are
        # AG-independent and stream inside the collective window (Pool SEQ
        # holds them until the AG dispatches). For L0 they are emitted
        # inline in phase P instead (interleaved with src gathers) so they
        # don't congest phase A's flush traffic.
        edc_f = sb_i.tile([P, TTmax * 4], BF16, tag="edc")
        edc = edc_f[:, 0:pl["T"] * H].rearrange("p (s h) -> p s h", h=H)
        last_ed = None

        def emit_ed(k0b, k1b, soff, Lsb):
            nonlocal last_ed
            if li == 0:
                deps = {id(bar_local): bar_local}
            else:
                deps = {id(prev_hw_byblock[k]): prev_hw_byblock[k]
                        for k in range(k0b, k1b) if k in prev_hw_byblock}
            et = sbg.tile([P, Lmax, P], BF16, tag="e")
            for cc0 in range(0, Lsb, CALL_SLOTS):
                cs = min(CALL_SLOTS, Lsb - cc0)
                gi = nc.gpsimd.dma_gather(
                    out_ap=et[:, cc0:cc0 + cs, :], in_ap=ed_src,
                    idxs_ap=c_ie[:, 8 * (soff + cc0):8 * (soff + cc0 + cs)],
                    num_idxs=cs * P, num_idxs_reg=cs * P,
                    elem_size=P, elem_step=roww,
                    single_packet=SINGLE_PACKET, queue_num=next_q())
                for dw in deps.values():
                    add_dep_helper(gi.ins, dw.ins, reason="ed gather dep")
                last_ed = gi
            # compact on ACT: DVE is the binding engine in phase P
            return nc.scalar.copy(
                out=edc[:, soff:soff + Lsb, :],
                in_=et[:, 0:Lsb, edo:edo + H])

        compacts = {}
        if li > 0:
            for (k0b, k1b, soff, Llo, Lhi) in pl["sb"]:
                compacts[soff] = emit_ed(k0b, k1b, soff, Llo + Lhi)

        if li == 0:
            # full-table barrier: src gathers touch all 50k rotated rows.
            bar0 = nc.gpsimd.engine_nop()
            for w in tbl0_writes:
                add_dep_helper(bar0.ins, w.ins, reason="tbl0 barrier")
            src_dep = bar0

        def flush_hown(stage, blocks, li2):
            cols2 = USED[li2]
            full = [(j, k) for (j, k, nk) in blocks if nk == P]
            part = [(j, k, nk) for (j, k, nk) in blocks if nk != P]
            ws = []
            if full:
                j0, k0 = full[0]
                cnt = len(full)
                w = nc.sync.dma_start(
                    out=howns[li2][k0 * P:(k0 + cnt) * P, 0:cols2].rearrange(
                        "(a p) c -> p a c", p=P),
                    in_=stage[:, j0:j0 + cnt, 0:cols2])
                ws.append(w)
            for (j, k, nk) in part:
                w = nc.sync.dma_start(
                    out=howns[li2][k * P:k * P + nk, 0:cols2],
                    in_=stage[:nk, j, 0:cols2])
                ws.append(w)
            return ws

        B_H = 4
        hstage = None
        hstage_blocks = []
        B_O = 8
        ostage = None
        ostage_blocks = []

        def _call(out3, o0, in_ap, idxt, ioff, cnt, elem, estep=None,
                  dep=None):
            for cc0 in range(0, cnt, CALL_SLOTS):
                cs = min(CALL_SLOTS, cnt - cc0)
                gi = nc.gpsimd.dma_gather(
                    out_ap=out3[:, o0 + cc0:o0 + cc0 + cs, :],
                    in_ap=in_ap,
                    idxs_ap=idxt[:, 8 * (ioff + cc0):8 * (ioff + cc0 + cs)],
                    num_idxs=cs * P, num_idxs_reg=cs * P,
                    elem_size=elem, elem_step=estep,
                    single_packet=SINGLE_PACKET, queue_num=next_q())
                add_dep_helper(
                    gi.ins, (dep if dep is not None else src_dep).ins,
                    reason="gather after producer")
                if last_ed is not None:
                    # keep all phase-E ed gathers ahead of src gathers in
                    # the DMASW lane rotation: a src gather scheduled onto
                    # a lane BEFORE an ed gather makes the ed gather's
                    # lane-wait transitively include the AG
                    add_dep_helper(gi.ins, last_ed.ins,
                                   reason="src after ed lanes")

        for sbi, (k0b, k1b, soff, Llo, Lhi) in enumerate(pl["sb"]):
            Lsb = Llo + Lhi
            nb = k1b - k0b

            if li == 0:
                emit_ed(k0b, k1b, soff, Lsb)

            g_fl = sbg.tile([P, Lmax * ROWW[0]], BF16, tag="g")
            gt = g_fl[:].rearrange("p (s r) -> p s r", r=roww)
            _call(gt, 0, tbl[li][:], c_il, olo[k0b], Llo, roww)
            _call(gt, Llo, tbl[li][SPLIT:, :], c_ih, ohi[k0b], Lhi, roww)

            es_sl = gt[:, 0:Lsb, es0:es0 + H]
            ed_sl = edc[:, soff:soff + Lsb, :]
            h_sl = gt[:, 0:Lsb, 0:OC]

            al_fl = sb.tile([P, Lmax * 4], BF16, tag="al")
            al = al_fl[:].rearrange("p (s h) -> p s h", h=H)
            ali = nc.vector.tensor_tensor(out=al[:, 0:Lsb, :], in0=es_sl,
                                          in1=ed_sl, op=mybir.AluOpType.add)
            if li > 0:
                # keep the phase-E compacts ahead of phase-P DVE work: an
                # AG-gated al op scheduled between compacts head-of-line
                # blocks them, back-pressuring the ed gathers (et-tile WAR)
                add_dep_helper(ali.ins, compacts[soff].ins,
                               reason="phase P after compacts")
            # leaky-relu and exp in place (SBUF is tight)
            nc.vector.scalar_tensor_tensor(
                out=al[:, 0:Lsb, :], in0=al[:, 0:Lsb, :], scalar=NEG,
                op0=mybir.AluOpType.mult, in1=al[:, 0:Lsb, :],
                op1=mybir.AluOpType.max)
            ea = al
            nc.scalar.activation(out=ea[:, 0:Lsb, :], in_=al[:, 0:Lsb, :],
                                 func=mybir.ActivationFunctionType.Exp)

            rhs_fl = sb.tile([P, (Lmax + SB) * 260], BF16, tag="rhs")
            rhs = rhs_fl[:, 0:(Lsb + nb) * RC].rearrange(
                "p (s c) -> p s c", c=RC)
            if Lsb:
                nc.vector.tensor_tensor(
                    out=rhs[:, 0:Lsb, 0:OC].rearrange(
                        "p s (h o) -> p s h o", o=O),
                    in0=h_sl.rearrange("p s (h o) -> p s h o", o=O),
                    in1=ea[:, 0:Lsb, :].unsqueeze(3).to_broadcast(
                        [P, Lsb, H, O]),
                    op=mybir.AluOpType.mult)
                nc.vector.tensor_copy(out=rhs[:, 0:Lsb, OC:RC],
                                      in_=ea[:, 0:Lsb, :])

            m = sb.tile([P, Lmax * P], BF16, tag="m")
            mv = m[:].rearrange("p (s n) -> p s n", n=P)
            if Lsb:
                # one-hot build on DVE (walrus rejects is_equal on Pool)
                meng = nc.vector
                meng.tensor_tensor(
                    out=mv[:, 0:Lsb, :],
                    in0=c_dl[:, soff:soff + Lsb].unsqueeze(2).to_broadcast(
                        [P, Lsb, P]),
                    in1=c_iota[:].unsqueeze(1).to_broadcast([P, Lsb, P]),
                    op=mybir.AluOpType.is_equal)

            # self rows for this superblock's blocks, batched
            hbs = sb.tile([P, SB * USED[0]], BF16, tag="hbs")
            hbv = hbs[:, 0:nb * cols_u].rearrange("p (s c) -> p s c",
                                                  c=cols_u)
            sdeps = ([bar_local] if li == 0 else
                     [prev_hw_byblock[k] for k in range(k0b, k1b)
                      if k in prev_hw_byblock])
            nfb = min(k1b, nfull) - k0b
            if nfb > 0:
                w = nc.sync.dma_start(
                    out=hbv[:, 0:nfb, :],
                    in_=self_src[k0b * P:(k0b + nfb) * P,
                                 0:cols_u].rearrange("(a p) c -> p a c", p=P))
                for dd in sdeps:
                    add_dep_helper(w.ins, dd.ins, reason="self rows dep")
            if k1b > nfull:
                j = nfull - k0b
                # verifier rejects partition-offset memsets; clear the whole
                # block column, the partial load then overwrites rows 0:rem
                nc.vector.memset(hbv[:, j, :], 0.0)
                w = nc.sync.dma_start(out=hbv[:rem, j, :],
                                      in_=self_src[nfull * P:NPD, 0:cols_u])
                for dd in sdeps:
                    add_dep_helper(w.ins, dd.ins, reason="self rows dep")
            asl_s = sb.tile([P, SB * 4], BF16, tag="asls")
            aslv = asl_s[:, 0:nb * H].rearrange("p (s h) -> p s h", h=H)
            nc.vector.tensor_tensor(out=aslv[:], in0=hbv[:, :, es0:es0 + H],
                                    in1=hbv[:, :, es0 + H:es0 + 2 * H],
                                    op=mybir.AluOpType.add)
            nc.vector.scalar_tensor_tensor(
                out=aslv[:], in0=aslv[:], scalar=NEG,
                op0=mybir.AluOpType.mult, in1=aslv[:],
                op1=mybir.AluOpType.max)
            nc.scalar.activation(out=aslv[:], in_=aslv[:],
                                 func=mybir.ActivationFunctionType.Exp)
            # self slots for all nb blocks, batched
            nc.vector.tensor_tensor(
                out=rhs[:, Lsb:Lsb + nb, 0:OC].rearrange(
                    "p s (h o) -> p s h o", o=O),
                in0=hbv[:, :, 0:OC].rearrange("p s (h o) -> p s h o", o=O),
                in1=aslv[:].unsqueeze(3).to_broadcast([P, nb, H, O]),
                op=mybir.AluOpType.mult)
            nc.vector.tensor_copy(out=rhs[:, Lsb:Lsb + nb, OC:RC],
                                  in_=aslv[:])

            for b in range(nb):
                k = k0b + b
                nk = min(P, NPD - k * P)
                slo, shi = S_lo[k], S_hi[k]
                sidx = Lsb + b

                u_ps = ps_u.tile([P, RC], F32, space="PSUM", tag="u")
                ranges = []
                if slo:
                    p0 = blk_lo[k] - soff
                    ranges.append((p0, p0 + slo))
                if shi:
                    p0 = blk_hi[k] - soff
                    ranges.append((p0, p0 + shi))
                first = True
                for (r0, r1) in ranges:
                    for j in range(r0, r1):
                        nc.tensor.matmul(
                            out=u_ps[:], lhsT=m[:, j * P:(j + 1) * P],
                            rhs=rhs[:, j, :], start=first, stop=False)
                        first = False
                nc.tensor.matmul(out=u_ps[:], lhsT=c_identb[:],
                                 rhs=rhs[:, sidx, :], start=first, stop=True)

                rec = sb.tile([P, 4], F32, tag="rec")
                nc.vector.reciprocal(out=rec[:, 0:H], in_=u_ps[:, OC:RC])
                obb = sb.tile([P, 256], F32, tag="obb")
                nc.vector.tensor_tensor(
                    out=obb[:, 0:OC].rearrange("p (h o) -> p h o", o=O),
                    in0=u_ps[:, 0:OC].rearrange("p (h o) -> p h o", o=O),
                    in1=rec[:, 0:H].unsqueeze(2).to_broadcast([P, H, O]),
                    op=mybir.AluOpType.mult)
                if not plans["bzero"][li]:
                    nc.vector.tensor_tensor(out=obb[:, 0:OC],
                                            in0=obb[:, 0:OC],
                                            in1=c_b[li][:],
                                            op=mybir.AluOpType.add)

                if last:
                    if ostage is None:
                        ostage = sbs.tile([P, B_O, 2], F32, tag="ostage")
                        ostage_blocks = []
                    nc.scalar.activation(
                        out=ostage[:nk, k % B_O, :], in_=obb[:nk, 0:2],
                        func=mybir.ActivationFunctionType.Relu)
                    ostage_blocks.append((k % B_O, k, nk))
                    if len(ostage_blocks) == B_O or k == NBLK - 1:
                        full = [(j, kk) for (j, kk, nn) in ostage_blocks
                                if nn == P]
                        part = [(j, kk, nn) for (j, kk, nn) in ostage_blocks
                                if nn != P]
                        if full:
                            j0, k0 = full[0]
                            nc.sync.dma_start(
                                out=out_own[k0 * P:(k0 + len(full)) * P, :]
                                    .rearrange("(a p) c -> p a c", p=P),
                                in_=ostage[:, j0:j0 + len(full), :])
                        for (j, kk, nn) in part:
                            nc.sync.dma_start(
                                out=out_own[kk * P:kk * P + nn, :],
                                in_=ostage[:nn, j, :])
                        ostage = None
                else:
                    orl = sb.tile([P, 256], F32, tag="orl")
                    nc.scalar.activation(
                        out=orl[:], in_=obb[:, 0:OC],
                        func=mybir.ActivationFunctionType.Relu)
                    li2 = li + 1
                    cols2 = USED[li2]
                    h2_ps = ps.tile([P, max(cols2, 8)], F32, space="PSUM",
                                    tag="h2ps")
                    nf = OC // P
                    for f in range(nf):
                        tp_ps = ps.tile([P, P], F32, space="PSUM", tag="tp")
                        nc.tensor.transpose(
                            out=tp_ps[:], in_=orl[:, f * P:(f + 1) * P],
                            identity=c_identf[:])
                        xt = sb.tile([P, P], F32R, tag=f"xt{f}")
                        nc.scalar.copy(out=xt[:], in_=tp_ps[:])
                        nc.tensor.matmul(
                            out=h2_ps[:, 0:cols2], lhsT=xt[:],
                            rhs=wprime[li2][f][:, 0:cols2],
                            start=(f == 0), stop=(f == nf - 1))
                    if hstage is None:
                        hstage = sbs.tile([P, B_H * USED[1]], BF16,
                                          tag="hstage")
                        hstage_blocks = []
                    hsv = hstage[:].rearrange("p (s c) -> p s c", c=cols2)
                    nc.scalar.copy(out=hsv[:nk, k % B_H, :],
                                   in_=h2_ps[:nk, 0:cols2])
                    hstage_blocks.append((k % B_H, k, nk))
                    if len(hstage_blocks) == B_H or k == NBLK - 1:
                        ws = flush_hown(hsv, hstage_blocks, li2)
                        for w in ws:
                            h_writes.append(w)
                            for (_, kk, _) in hstage_blocks:
                                hw_byblock[kk] = w
                        hstage = None


# --------------------------------------------------------------------------
# entry point
# --------------------------------------------------------------------------

_cache = {}
TRACE = False
last_result = None


def _plan_key(plans):
    return (plans["A"]["S_lo"], plans["A"]["S_hi"],
            plans["B"]["S_lo"], plans["B"]["S_hi"], plans["bzero"])


def kernel(x, edge_index, W0, a_src0, a_dst0, b0, W1, a_src1, a_dst1, b1,
           W2, a_src2, a_dst2, b2):
    weights = [(W0, a_src0, a_dst0, b0), (W1, a_src1, a_dst1, b1),
               (W2, a_src2, a_dst2, b2)]
    in_maps, plans = _host_prep(np.asarray(x), np.asarray(edge_index), weights)

    key = _plan_key(plans)
    if key not in _cache:
        _cache[key] = build_program(plans)
    nc = _cache[key]

    global last_result
    res = run_bass_kernel_spmd(nc, in_maps, core_ids=list(range(GEO.NCORES)),
                               trace=TRACE)
    last_result = res
    out = np.concatenate(
        [res.results[d]["out"] for d in range(GEO.NCORES)], axis=0)
    return out.astype(np.float32)



# revision 69
# speedup vs baseline: 1.0557x; 1.0557x over previous
"""Trainium2 Bass kernel for the 3-layer dual-head GAT (nn_DualHeadGAT), v2.

Strategy (dst-range sharded edges, bf16 tables, no layer-0 AllGather):
  - Nodes split contiguously across 8 cores (6250 each); edges sorted by
    destination so all segment reductions are core-local.
  - Per-node feature tables are bf16 rows [h | es | ed | pad]:
    layers 0/1: 384 cols (768B rows), layer 2: 128 cols (256B rows).
  - Layer 0's table is computed IN FULL on every core from x (x is tiny),
    laid out *rotated* so each core's own nodes come first; this removes
    the layer-0 AllGather. Layers 1/2 AllGather bf16 tables.
  - Edges are processed in superblocks of SB dst-blocks: one dma_gather
    per (lo, hi, ed) region per superblock (Pool-engine SWDGE call
    overhead is ~1us each), and all per-edge elementwise math runs as one
    instruction per superblock.  Per edge, the src row (h+es) is gathered
    by rotated/global src id (int16 split at 32768); the dst row slice
    (ed) is gathered from the core-local table with local dst ids.
  - Scatter-add per 128-dst-node block via one-hot matmul on the PE:
      U = sum_j M_j^T @ [ea*h | ea],  M[e, n] = (dst_local[e] == n)
    with M and rhs in bf16 (PSUM accumulates f32). Self-loops are folded
    in as one extra identity-matrix slot per block.
  - Per-node normalize: out = relu(U[:, :OC]/U[:, OC:] + b); next-layer
    rows h2 = out @ W' via PE transpose + f32r matmul (full precision
    weights), written bf16.

Self-contained: hardcodes the problem shapes; host-side preprocessing of
edge_index is pure index manipulation. All float math runs on device.
"""
import math
from contextlib import ExitStack
import numpy as np
import ml_dtypes

import concourse.bass as bass
import concourse.bacc as bacc
import concourse.mybir as mybir
import concourse.tile as tile
from concourse.bass_utils import run_bass_kernel_spmd
from concourse.tile_rust import add_dep_helper

F32 = mybir.dt.float32
F32R = mybir.dt.float32r
BF16 = mybir.dt.bfloat16
I16 = mybir.dt.int16
BF_NP = ml_dtypes.bfloat16

P = 128
NEG = 0.2


class Geo:
    def __init__(self, n=50000, ncores=8, split=32768):
        self.N = n
        self.NCORES = ncores
        self.NPD = n // ncores
        self.NBLK = math.ceil(self.NPD / P)
        self.NBLK_ALL = math.ceil(n / P)
        self.SPLIT = split


GEO = Geo()

LAYERS = [(2, 4, 64), (256, 4, 64), (256, 1, 2)]
ROWW = [384, 384, 128]     # bf16 row width of each layer's table
USED = [264, 264, 4]       # h | es | ed used cols
ES0 = [256, 256, 2]        # es col offset within row

SB = 2                     # dst-blocks per gather superblock
CALL_SLOTS = 8            # max 128-edge slots per dma_gather call
SCRATCH = 16384           # SWDGE ring bytes (ring descs = SCRATCH//16)
SINGLE_PACKET = True      # multi-packet SWDGE for big calls
QROT = 4                   # rotate gather calls over this many SWDGE queues
                           # (sim's sem-lane/queue affinity check requires 1)
STAGE = 0                  # 0=full, 1=phaseA only, 2=+L0, 3=+AG1+L1


# --------------------------------------------------------------------------
# host preprocessing
# --------------------------------------------------------------------------

def _wrap16(seq):
    """dma_gather index layout: idx i lives at [i % 16, i // 16]; tiled to
    128 partitions."""
    seq = np.asarray(seq, np.int16)
    a = seq.reshape(-1, 16).T
    return np.tile(a, (8, 1))


def _build_struct(src_key, dst_all, geo, sbsz):
    """Slot structure for one gather-key function, superblock-ordered.

    Slot layout: per superblock of sbsz dst-blocks:
    [lo(k0) lo(k1) ... hi(k0) hi(k1) ...]."""
    N, NCORES, NPD, NBLK, SPLIT = (geo.N, geo.NCORES, geo.NPD, geo.NBLK,
                                   geo.SPLIT)
    cnt_lo = np.zeros((NCORES, NBLK), np.int64)
    cnt_hi = np.zeros((NCORES, NBLK), np.int64)
    parts = {}
    for d in range(NCORES):
        for k in range(NBLK):
            i = d * NBLK + k
            keys = src_key(d, i)
            dsts = dst_all[i]
            lo = keys < SPLIT
            parts[(d, k)] = (keys[lo], dsts[lo], keys[~lo] - SPLIT, dsts[~lo])
            cnt_lo[d, k] = int(lo.sum())
            cnt_hi[d, k] = int((~lo).sum())

    S_lo = np.ceil(cnt_lo.max(axis=0) / P).astype(np.int64)
    S_hi = np.ceil(cnt_hi.max(axis=0) / P).astype(np.int64)
    olo = np.concatenate([[0], np.cumsum(S_lo)]).astype(np.int64)
    ohi = np.concatenate([[0], np.cumsum(S_hi)]).astype(np.int64)
    T_lo, T_hi = int(olo[-1]), int(ohi[-1])

    blk_lo_pos = np.zeros(NBLK, np.int64)
    blk_hi_pos = np.zeros(NBLK, np.int64)
    sb_bounds = []
    pos = 0
    for k0 in range(0, NBLK, sbsz):
        k1 = min(k0 + sbsz, NBLK)
        Llo = int(S_lo[k0:k1].sum())
        Lhi = int(S_hi[k0:k1].sum())
        pp = pos
        for k in range(k0, k1):
            blk_lo_pos[k] = pp
            pp += S_lo[k]
        for k in range(k0, k1):
            blk_hi_pos[k] = pp
            pp += S_hi[k]
        sb_bounds.append((k0, k1, pos, Llo, Lhi))
        pos += Llo + Lhi
    T = pos

    tabs = []
    for d in range(NCORES):
        idx_lo = np.zeros((P, 8 * max(T_lo, 1)), np.int16)
        idx_hi = np.zeros((P, 8 * max(T_hi, 1)), np.int16)
        idx_ed = np.zeros((P, 8 * max(T, 1)), np.int16)
        dloc = np.full((P, max(T, 1)), 999.0, BF_NP)

        def place(pos0, nslots, keys, dsts, base, tab):
            if nslots == 0:
                return
            buf = np.zeros(nslots * P, np.int16)
            buf[:len(keys)] = keys.astype(np.int16)
            tab[:, 8 * pos0:8 * (pos0 + nslots)] = _wrap16(buf)

        for k in range(NBLK):
            kl, dl, kh, dh = parts[(d, k)]
            slo, shi = int(S_lo[k]), int(S_hi[k])
            base = d * NPD + k * P
            place(int(olo[k]), slo, kl, dl, base, idx_lo)
            place(int(ohi[k]), shi, kh, dh, base, idx_hi)

            for pos0, nsl, dd in ((int(blk_lo_pos[k]), slo, dl),
                                  (int(blk_hi_pos[k]), shi, dh)):
                if nsl == 0:
                    continue
                buf = np.zeros(nsl * P, np.int16)
                buf[:len(dd)] = (dd - d * NPD).astype(np.int16)
                idx_ed[:, 8 * pos0:8 * (pos0 + nsl)] = _wrap16(buf)
                loc = np.full(nsl * P, 999.0, np.float32)
                loc[:len(dd)] = (dd - base).astype(np.float32)
                dloc[:, pos0:pos0 + nsl] = \
                    loc.reshape(nsl, P).T.astype(BF_NP)
        tabs.append((idx_lo, idx_hi, idx_ed, dloc))

    plan = {
        "S_lo": tuple(int(s) for s in S_lo),
        "S_hi": tuple(int(s) for s in S_hi),
        "olo": tuple(int(o) for o in olo),
        "ohi": tuple(int(o) for o in ohi),
        "blk_lo": tuple(int(v) for v in blk_lo_pos),
        "blk_hi": tuple(int(v) for v in blk_hi_pos),
        "sb": tuple(sb_bounds),
        "T": T, "T_lo": T_lo, "T_hi": T_hi,
    }
    return plan, tabs


def _host_prep(x, edge_index, weights, geo=GEO):
    N, NCORES, NPD, NBLK = geo.N, geo.NCORES, geo.NPD, geo.NBLK
    src = np.asarray(edge_index[0]).astype(np.int64)
    dst = np.asarray(edge_index[1]).astype(np.int64)
    perm = np.argsort(dst, kind="stable")
    s_src = src[perm].astype(np.int64)
    s_dst = dst[perm].astype(np.int64)

    starts, stops = [], []
    for d in range(NCORES):
        for k in range(NBLK):
            starts.append(d * NPD + k * P)
            stops.append(min(d * NPD + (k + 1) * P, (d + 1) * NPD))
    e_lo = np.searchsorted(s_dst, starts)
    e_hi = np.searchsorted(s_dst, stops)
    src_blk = {i: s_src[e_lo[i]:e_hi[i]] for i in range(NCORES * NBLK)}
    dst_blk = {i: s_dst[e_lo[i]:e_hi[i]] for i in range(NCORES * NBLK)}

    # structure A: layer-0 rotated keys; structure B: global keys
    planA, tabsA = _build_struct(
        lambda d, i: (src_blk[i] - d * NPD) % N, dst_blk, geo, SB)
    planB, tabsB = _build_struct(
        lambda d, i: src_blk[i], dst_blk, geo, SB)

    x = np.asarray(x, np.float32)
    bzero = tuple(bool(np.all(np.asarray(w[3]) == 0)) for w in weights)
    in_maps = []
    for d in range(NCORES):
        ilA, ihA, ieA, dlA = tabsA[d]
        ilB, ihB, ieB, dlB = tabsB[d]
        rot = np.roll(np.arange(N), -d * NPD)
        m = {
            "xTr": np.ascontiguousarray(x[rot].T),
            "idx_loA": ilA, "idx_hiA": ihA, "idx_edA": ieA, "dlocA": dlA,
            "idx_loB": ilB, "idx_hiB": ihB, "idx_edB": ieB, "dlocB": dlB,
            "iota_bf": np.tile(np.arange(P, dtype=BF_NP)[None, :], (P, 1)),
            "ident_bf": np.eye(P, dtype=BF_NP),
            "ident_f": np.eye(P, dtype=np.float32),
        }
        for li, (W, a_s, a_d, b) in enumerate(weights):
            fin, H, O = LAYERS[li]
            W = np.asarray(W, np.float32)
            a_s = np.asarray(a_s, np.float32)
            a_d = np.asarray(a_d, np.float32)
            b = np.asarray(b, np.float32)
            As = np.zeros((H * O, H), np.float32)
            Ad = np.zeros((H * O, H), np.float32)
            for h in range(H):
                As[h * O:(h + 1) * O, h] = a_s[h]
                Ad[h * O:(h + 1) * O, h] = a_d[h]
            m[f"W{li}"] = W
            m[f"WT{li}"] = np.ascontiguousarray(W.T)
            m[f"As{li}"] = As
            m[f"Ad{li}"] = Ad
            m[f"b{li}"] = np.tile(b[None, :], (P, 1))
        in_maps.append(m)

    return in_maps, {"A": planA, "B": planB, "bzero": bzero}


# --------------------------------------------------------------------------
# device program
# --------------------------------------------------------------------------

def build_program(plans, geo=GEO, repeat=1):
    N, NCORES, NPD = geo.N, geo.NCORES, geo.NPD
    nc = bacc.Bacc("TRN2", target_bir_lowering=False, debug=False,
                   num_devices=NCORES, num_swdge_queues=4,
                   dynamic_dma_scratch_size=SCRATCH)

    t_in = {}

    def inp(name, shape, dt=F32):
        t_in[name] = nc.dram_tensor(name, shape, dt, kind="ExternalInput").ap()

    pA, pB = plans["A"], plans["B"]
    inp("xTr", [2, N])
    inp("idx_loA", [P, 8 * max(pA["T_lo"], 1)], I16)
    inp("idx_hiA", [P, 8 * max(pA["T_hi"], 1)], I16)
    inp("idx_edA", [P, 8 * max(pA["T"], 1)], I16)
    inp("dlocA", [P, max(pA["T"], 1)], BF16)
    inp("idx_loB", [P, 8 * max(pB["T_lo"], 1)], I16)
    inp("idx_hiB", [P, 8 * max(pB["T_hi"], 1)], I16)
    inp("idx_edB", [P, 8 * max(pB["T"], 1)], I16)
    inp("dlocB", [P, max(pB["T"], 1)], BF16)
    inp("iota_bf", [P, P], BF16)
    inp("ident_bf", [P, P], BF16)
    inp("ident_f", [P, P])
    for li, (fin, H, O) in enumerate(LAYERS):
        OC = H * O
        inp(f"W{li}", [fin, OC])
        inp(f"WT{li}", [OC, fin])
        inp(f"As{li}", [OC, H])
        inp(f"Ad{li}", [OC, H])
        inp(f"b{li}", [P, OC])

    out_own = nc.dram_tensor("out", [NPD, 2], F32, kind="ExternalOutput").ap()

    tbl = [nc.dram_tensor("tbl0", [N, ROWW[0]], BF16, kind="Internal").ap()]
    for li in (1, 2):
        tbl.append(nc.dram_tensor(f"tbl{li}", [N, ROWW[li]], BF16,
                                  kind="Internal", addr_space="Shared").ap())
    howns = [None,
             nc.dram_tensor("hown1", [NPD, ROWW[1]], BF16, kind="Internal").ap(),
             nc.dram_tensor("hown2", [NPD, ROWW[2]], BF16, kind="Internal").ap()]
    aux = {
        "hown2p": nc.dram_tensor("hown2p", [NPD, 4], BF16,
                                 kind="Internal").ap(),
        "tbl2p": nc.dram_tensor("tbl2p", [N, 4], BF16, kind="Internal",
                                addr_space="Shared").ap(),
    }

    with tile.TileContext(nc) as tc:
        _emit(tc, t_in, out_own, tbl, howns, aux, plans, geo, repeat)

    # Post-scheduling: pin each gather's SWDGE queue to its assigned DMASW
    # sem lane (lane rotates per Pool-DMA inst in scheduled order; the sem
    # of lane L must always be updated from one queue, so queue = L % QROT).
    if QROT > 1:
        for blk in nc.m.functions[0].blocks:
            for inst in blk.instructions:
                if isinstance(inst, mybir.InstDMAGatherAnt):
                    proc = getattr(inst, "bass_scheduled_proc", None)
                    if proc is not None and 11 <= proc <= 18:
                        inst.queue_num = (proc - 11) % QROT
    nc.compile()
    return nc


def _emit(tc, t_in, out_own, tbl, howns, aux, plans, geo, repeat):
    nc = tc.nc
    pA, pB = plans["A"], plans["B"]
    Lmax = max(lo + hi for pl in (pA, pB) for (_, _, _, lo, hi) in pl["sb"])

    ctx = ExitStack()
    sb_c = ctx.enter_context(tc.tile_pool(name="const", bufs=1))
    sb_i = ctx.enter_context(tc.tile_pool(name="idx", bufs=1))
    sb = ctx.enter_context(tc.tile_pool(name="work", bufs=2))
    sbg = ctx.enter_context(tc.tile_pool(name="gath", bufs=2))
    sbs = ctx.enter_context(tc.tile_pool(name="stage", bufs=2))
    ps = ctx.enter_context(tc.tile_pool(name="psum", bufs=2, space="PSUM"))
    ps_u = ctx.enter_context(tc.tile_pool(name="psum_u", bufs=2, space="PSUM"))

    # ---- persistent constants ----
    def load_const(name, shape, dt=F32):
        t = sb_c.tile(shape, dt, tag=name)
        nc.sync.dma_start(out=t[:], in_=t_in[name][:])
        return t

    c_iota = load_const("iota_bf", [P, P], BF16)
    c_identb = load_const("ident_bf", [P, P], BF16)
    c_identf = load_const("ident_f", [P, P])
    c_b = [load_const(f"b{li}", [P, LAYERS[li][1] * LAYERS[li][2]])
           for li in range(3)]

    # ---- per-structure gather index tiles (one structure resident) ----
    TLO = max(pA["T_lo"], pB["T_lo"], 1)
    THI = max(pA["T_hi"], pB["T_hi"], 1)
    TT = max(pA["T"], pB["T"], 1)

    def load_struct(sfx):
        p = pA if sfx == "A" else pB
        il = sb_i.tile([P, 8 * TLO], I16, tag="il")
        ih = sb_i.tile([P, 8 * THI], I16, tag="ih")
        ie = sb_i.tile([P, 8 * TT], I16, tag="ie")
        dl = sb_i.tile([P, TT], BF16, tag="dl")
        nc.sync.dma_start(out=il[:, 0:8 * max(p["T_lo"], 1)],
                          in_=t_in[f"idx_lo{sfx}"][:])
        nc.sync.dma_start(out=ih[:, 0:8 * max(p["T_hi"], 1)],
                          in_=t_in[f"idx_hi{sfx}"][:])
        nc.sync.dma_start(out=ie[:, 0:8 * max(p["T"], 1)],
                          in_=t_in[f"idx_ed{sfx}"][:])
        nc.sync.dma_start(out=dl[:, 0:max(p["T"], 1)],
                          in_=t_in[f"dloc{sfx}"][:])
        return il, ih, ie, dl

    # ---- W' = [W | W@As | W@Ad] per layer (f32, tiny) ----
    wprime = []
    for li, (fin, H, O) in enumerate(LAYERS):
        OC = H * O
        n_fin_t = math.ceil(fin / P)
        n_k_t = math.ceil(OC / P)
        kp = min(P, OC)
        tiles = []
        for fi in range(n_fin_t):
            fr = min(P, fin - fi * P)
            wp = sb_c.tile([P, OC + 2 * H], F32R, tag=f"wp{li}_{fi}")
            nc.sync.dma_start(out=wp[:fr, 0:OC],
                              in_=t_in[f"W{li}"][fi * P:fi * P + fr, :]
                              .bitcast(F32R))
            for ci, aname in ((0, f"As{li}"), (1, f"Ad{li}")):
                wa_fl = ps.tile([P, 264], F32, space="PSUM", tag="h0ps")
                wa_ps = wa_fl[:, 0:H]
                a_sb = sb.tile([P, n_k_t, H], F32, tag="a_in")
                nc.sync.dma_start(
                    out=a_sb[:kp, 0:n_k_t, :],
                    in_=t_in[aname][:].rearrange("(a p) h -> p a h", p=kp))
                wt_sb = sb.tile([P, n_k_t, P], F32, tag="wt_in")
                nc.sync.dma_start(
                    out=wt_sb[:kp, 0:n_k_t, 0:fr],
                    in_=t_in[f"WT{li}"][:, fi * P:fi * P + fr].rearrange(
                        "(a p) f -> p a f", p=kp))
                for ki in range(n_k_t):
                    kr = min(P, OC - ki * P)
                    nc.tensor.matmul(
                        out=wa_ps[:fr, :],
                        lhsT=wt_sb[:kr, ki, 0:fr],
                        rhs=a_sb[:kr, ki, :],
                        start=(ki == 0), stop=(ki == n_k_t - 1))
                nc.vector.tensor_copy(
                    out=wp[:fr, OC + ci * H:OC + (ci + 1) * H],
                    in_=wa_ps[:fr, :])
            tiles.append(wp)
        wprime.append(tiles)

    consts = (c_iota, c_identb, c_identf, c_b, wprime, load_struct)
    pools = (sb, sbg, sbs, ps, ps_u, sb_i)
    for rep in range(repeat):
        _emit_iter(tc, t_in, out_own, tbl, howns, aux, plans, geo, consts,
                   pools, Lmax)

    ctx.close()


def _emit_iter(tc, t_in, out_own, tbl, howns, aux, plans, geo, consts, pools,
               Lmax):
    nc = tc.nc
    N, NCORES, NPD, NBLK, NBLK_ALL, SPLIT = (geo.N, geo.NCORES, geo.NPD,
                                             geo.NBLK, geo.NBLK_ALL, geo.SPLIT)
    (c_iota, c_identb, c_identf, c_b, wprime, load_struct) = consts
    sb, sbg, sbs, ps, ps_u, sb_i = pools
    pA, pB = plans["A"], plans["B"]

    qctr = [0]
    TTmax = max(plans["A"]["T"], plans["B"]["T"], 1)

    def next_q():
        return 0

    # ================= phase A: full layer-0 table (rotated) =================
    XCHUNK = 384  # blocks of x columns per SBUF load
    tbl0_writes = []
    tbl0_local_writes = []  # writes covering table rows [0:NPD] (own nodes)

    def flush_tbl0(stage, blocks):
        full = [(j, gg) for (j, gg, nk) in blocks if nk == P]
        part = [(j, gg, nk) for (j, gg, nk) in blocks if nk != P]
        ws = []
        if full:
            j0, g0 = full[0]
            cnt = len(full)
            w = nc.sync.dma_start(
                out=tbl[0][g0 * P:(g0 + cnt) * P, 0:264].rearrange(
                    "(a p) c -> p a c", p=P),
                in_=stage[:, j0:j0 + cnt, :])
            ws.append(w)
        for (j, gg, nk) in part:
            w = nc.sync.dma_start(
                out=tbl[0][gg * P:gg * P + nk, 0:264],
                in_=stage[:nk, j, :])
            ws.append(w)
        return ws

    B_ST = 6
    g = 0
    eng_rot = [nc.vector, nc.scalar]   # Pool cannot read PSUM
    while g < NBLK_ALL:
        c0 = g * P
        ccols = min(XCHUNK, N - c0)
        nblk_c = math.ceil(ccols / P)
        xc = sb.tile([2, XCHUNK], F32R, tag="xc")
        nc.sync.dma_start(out=xc[:2, 0:ccols],
                          in_=t_in["xTr"][:, c0:c0 + ccols].bitcast(F32R))
        done = 0
        while done < nblk_c:
            grp = min(B_ST, nblk_c - done)
            stage = sbs.tile([P, B_ST, 264], BF16, tag="stA")
            blocks = []
            for j in range(grp):
                gb = g + done + j
                nk = min(P, N - gb * P)
                h0_ps = ps.tile([P, 264], F32, space="PSUM", tag="h0ps")
                nc.tensor.matmul(
                    out=h0_ps[:nk, :],
                    lhsT=xc[:2, (done + j) * P:(done + j) * P + nk],
                    rhs=wprime[0][0][:2, 0:264],
                    start=True, stop=True)
                eng = eng_rot[gb % 2]
                if eng is nc.scalar:
                    nc.scalar.copy(out=stage[:nk, j, :], in_=h0_ps[:nk, :])
                else:
                    eng.tensor_copy(out=stage[:nk, j, :], in_=h0_ps[:nk, :])
                blocks.append((j, gb, nk))
            ws = flush_tbl0(stage, blocks)
            tbl0_writes += ws
            if any(gg < NBLK for (_, gg, _) in blocks):
                tbl0_local_writes += ws
            done += grp
        g += nblk_c

    # barrier over just the own-node rows [0:NPD] — unblocks ed gathers and
    # self-row loads long before the full table is written
    bar_local = nc.gpsimd.engine_nop()
    for w in tbl0_local_writes:
        add_dep_helper(bar_local.ins, w.ins, reason="tbl0 local barrier")

    if STAGE == 1:
        return

    # ================= layers =================
    h_writes = []
    hw_byblock = {}
    pkv = None   # persistent packed L2 rows [P, NBLK, 4] (built in L1)

    n_layers = {0: 3, 2: 1, 3: 2}[STAGE]
    for li in range(n_layers):
        fin, H, O = LAYERS[li]
        OC = H * O
        RC = OC + H
        roww = ROWW[li]
        es0 = ES0[li]
        # ed col offset inside the gathered 128-col ed row: layers 0/1 gather
        # row cols [256:384] (es at 0, ed at H); layer 2 gathers [0:128]
        edo = H if li < 2 else 3
        last = (li == 2)
        pl = pA if li == 0 else pB
        if li == 0:
            struct = load_struct("A")
        elif li == 1:
            struct = load_struct("B")
        c_il, c_ih, c_ie, c_dl = struct
        S_lo, S_hi = pl["S_lo"], pl["S_hi"]
        olo, ohi = pl["olo"], pl["ohi"]
        blk_lo, blk_hi = pl["blk_lo"], pl["blk_hi"]

        prev_h_writes = h_writes
        prev_hw_byblock = hw_byblock
        if li == 0:
            ed_src = tbl[0][0:NPD, 256:384]
            self_src = tbl[0]
        elif li == 1:
            ed_src = howns[1][:, 256:384]
            self_src = howns[1]
        else:
            ed_src = howns[2][:, 0:128]
            self_src = howns[2]
        hw_byblock = {}
        h_writes = []

        cols_u = USED[li]
        nfull = NPD // P
        rem = NPD - nfull * P

        # funnel the ~13 h-write DMAs through one nop: consumers that need
        # "all h writes done" get a single wait slot instead of 13 — with
        # too many waits the sem-assignment coarsens to "wait on the AG",
        # which silently serializes phase E behind the collective.
        hbar = None
        if li >= 1:
            hbar = nc.vector.engine_nop()
            for w in prev_h_writes:
                add_dep_helper(hbar.ins, w.ins, reason="h-writes funnel")

        # ---- collectives first: Pool's in-order SEQ then holds the ed
        # gathers (emitted next) until the AG *dispatches* (= h-writes
        # done), so they stream inside the collective window instead of
        # congesting the previous layer's tail.
        if li == 1:
            ag = nc.gpsimd.collective_compute(
                "AllGather", mybir.AluOpType.bypass,
                replica_groups=[list(range(NCORES))],
                ins=[howns[1][:]], outs=[tbl[1][:]],
            )
            add_dep_helper(ag.ins, hbar.ins, reason="AG after h writes")
            src_dep = ag
            # packed L2-row tile, filled by this layer's flushes
            pk_f = sb_i.tile([P, NBLK * 4], BF16, tag="pk2")
            pkv = pk_f[:].rearrange("p (s c) -> p s c", c=4)
            nc.vector.memset(pkv[:], 0.0)
        elif li == 2:
            # L2 rows use 4 cols — AllGather a packed [N, 4] table (0.4MB
            # vs 12.8MB). All DRAM traffic here is contiguous or full
            # 256B-row writes: 16B-strided HBM writes are RMW-bound on HW.
            w1 = nc.sync.dma_start(
                out=aux["hown2p"][0:nfull * P].rearrange(
                    "(a p) c -> p a c", p=P),
                in_=pkv[:, 0:nfull, :])
            w2 = nc.sync.dma_start(out=aux["hown2p"][nfull * P:NPD],
                                   in_=pkv[:rem, nfull, :])
            ag = nc.gpsimd.collective_compute(
                "AllGather", mybir.AluOpType.bypass,
                replica_groups=[list(range(NCORES))],
                ins=[aux["hown2p"][:]], outs=[aux["tbl2p"][:]],
            )
            add_dep_helper(ag.ins, w1.ins, reason="AG after pack")
            add_dep_helper(ag.ins, w2.ins, reason="AG after pack")
            # expand into the 128-col-stride gather table via SBUF bounce,
            # writing full 256B rows (cols 4:128 garbage, never read)
            ex_parts = []
            nblk_all = math.ceil(N / P)
            CH = 48
            b0 = 0
            while b0 < nblk_all:
                nb2 = min(CH, nblk_all - b0)
                r0 = b0 * P
                r1 = min(N, (b0 + nb2) * P)
                nf2 = (r1 - r0) // P
                bo_f = sbg.tile([P, Lmax * ROWW[0]], BF16, tag="g")
                st = bo_f[:, 0:CH * P].rearrange("p (s c) -> p s c", c=P)
                if nf2:
                    ld = nc.sync.dma_start(
                        out=st[:, 0:nf2, 0:4],
                        in_=aux["tbl2p"][r0:r0 + nf2 * P].rearrange(
                            "(a p) c -> p a c", p=P))
                    add_dep_helper(ld.ins, ag.ins, reason="expand ld")
                    wr = nc.sync.dma_start(
                        out=tbl[2][r0:r0 + nf2 * P, :].rearrange(
                            "(a p) c -> p a c", p=P),
                        in_=st[:, 0:nf2, :])
                    ex_parts.append(wr)
                if r1 > r0 + nf2 * P:
                    pr = r1 - r0 - nf2 * P
                    ld = nc.sync.dma_start(
                        out=st[:pr, min(nf2, CH - 1), 0:4],
                        in_=aux["tbl2p"][r0 + nf2 * P:r1])
                    add_dep_helper(ld.ins, ag.ins, reason="expand ld")
                    wr = nc.sync.dma_start(
                        out=tbl[2][r0 + nf2 * P:r1, :],
                        in_=st[:pr, min(nf2, CH - 1), :])
                    ex_parts.append(wr)
                b0 += nb2
            exbar = nc.vector.engine_nop()
            for w in ex_parts:
                add_dep_helper(exbar.ins, w.ins, reason="expand funnel")
            src_dep = exbar

        # ---- phase E: ed gathers compacted into edc. For L1/L2 these are
        # AG-independent and stream inside the collective window (Pool SEQ
        # holds them until the AG dispatches). For L0 they are emitted
        # inline in phase P instead (interleaved with src gathers) so they
        # don't congest phase A's flush traffic.
        edc_f = sb_i.tile([P, TTmax * 4], BF16, tag="edc")
        edc = edc_f[:, 0:pl["T"] * H].rearrange("p (s h) -> p s h", h=H)
        last_ed = None

        def emit_ed(k0b, k1b, soff, Lsb):
            nonlocal last_ed
            if li == 0:
                deps = {id(bar_local): bar_local}
            else:
                deps = {id(prev_hw_byblock[k]): prev_hw_byblock[k]
                        for k in range(k0b, k1b) if k in prev_hw_byblock}
            et = sbg.tile([P, Lmax, P], BF16, tag="e")
            for cc0 in range(0, Lsb, CALL_SLOTS):
                cs = min(CALL_SLOTS, Lsb - cc0)
                gi = nc.gpsimd.dma_gather(
                    out_ap=et[:, cc0:cc0 + cs, :], in_ap=ed_src,
                    idxs_ap=c_ie[:, 8 * (soff + cc0):8 * (soff + cc0 + cs)],
                    num_idxs=cs * P, num_idxs_reg=cs * P,
                    elem_size=P, elem_step=roww,
                    single_packet=SINGLE_PACKET, queue_num=next_q())
                for dw in deps.values():
                    add_dep_helper(gi.ins, dw.ins, reason="ed gather dep")
                last_ed = gi
            # compact on ACT: DVE is the binding engine in phase P
            return nc.scalar.copy(
                out=edc[:, soff:soff + Lsb, :],
                in_=et[:, 0:Lsb, edo:edo + H])

        compacts = {}
        if li > 0:
            for (k0b, k1b, soff, Llo, Lhi) in pl["sb"]:
                compacts[soff] = emit_ed(k0b, k1b, soff, Llo + Lhi)

        if li == 0:
            # full-table barrier: src gathers touch all 50k rotated rows.
            bar0 = nc.gpsimd.engine_nop()
            for w in tbl0_writes:
                add_dep_helper(bar0.ins, w.ins, reason="tbl0 barrier")
            src_dep = bar0

        def flush_hown(stage, blocks, li2):
            # L2 rows: write full 256B rows (cols 4:128 garbage) — 16B
            # strided HBM writes are RMW-bound on HW. Also mirror the 4
            # used cols into the packed SBUF tile pkv.
            wcols = USED[li2] if li2 == 1 else P
            full = [(j, k) for (j, k, nk) in blocks if nk == P]
            part = [(j, k, nk) for (j, k, nk) in blocks if nk != P]
            ws = []
            if full:
                j0, k0 = full[0]
                cnt = len(full)
                w = nc.sync.dma_start(
                    out=howns[li2][k0 * P:(k0 + cnt) * P, 0:wcols].rearrange(
                        "(a p) c -> p a c", p=P),
                    in_=stage[:, j0:j0 + cnt, 0:wcols])
                ws.append(w)
                if li2 == 2:
                    nc.vector.tensor_copy(out=pkv[:, k0:k0 + cnt, :],
                                          in_=stage[:, j0:j0 + cnt, 0:4])
            for (j, k, nk) in part:
                w = nc.sync.dma_start(
                    out=howns[li2][k * P:k * P + nk, 0:wcols],
                    in_=stage[:nk, j, 0:wcols])
                ws.append(w)
                if li2 == 2:
                    nc.vector.tensor_copy(out=pkv[:nk, k, :],
                                          in_=stage[:nk, j, 0:4])
            return ws

        B_H = 4
        hstage = None
        hstage_blocks = []
        B_O = 8
        ostage = None
        ostage_blocks = []

        def _call(out3, o0, in_ap, idxt, ioff, cnt, elem, estep=None,
                  dep=None):
            for cc0 in range(0, cnt, CALL_SLOTS):
                cs = min(CALL_SLOTS, cnt - cc0)
                gi = nc.gpsimd.dma_gather(
                    out_ap=out3[:, o0 + cc0:o0 + cc0 + cs, :],
                    in_ap=in_ap,
                    idxs_ap=idxt[:, 8 * (ioff + cc0):8 * (ioff + cc0 + cs)],
                    num_idxs=cs * P, num_idxs_reg=cs * P,
                    elem_size=elem, elem_step=estep,
                    single_packet=SINGLE_PACKET, queue_num=next_q())
                add_dep_helper(
                    gi.ins, (dep if dep is not None else src_dep).ins,
                    reason="gather after producer")
                if last_ed is not None:
                    # keep all phase-E ed gathers ahead of src gathers in
                    # the DMASW lane rotation: a src gather scheduled onto
                    # a lane BEFORE an ed gather makes the ed gather's
                    # lane-wait transitively include the AG
                    add_dep_helper(gi.ins, last_ed.ins,
                                   reason="src after ed lanes")

        for sbi, (k0b, k1b, soff, Llo, Lhi) in enumerate(pl["sb"]):
            Lsb = Llo + Lhi
            nb = k1b - k0b

            if li == 0:
                emit_ed(k0b, k1b, soff, Lsb)

            g_fl = sbg.tile([P, Lmax * ROWW[0]], BF16, tag="g")
            gt = g_fl[:].rearrange("p (s r) -> p s r", r=roww)
            _call(gt, 0, tbl[li][:], c_il, olo[k0b], Llo, roww)
            _call(gt, Llo, tbl[li][SPLIT:, :], c_ih, ohi[k0b], Lhi, roww)

            es_sl = gt[:, 0:Lsb, es0:es0 + H]
            ed_sl = edc[:, soff:soff + Lsb, :]
            h_sl = gt[:, 0:Lsb, 0:OC]

            al_fl = sb.tile([P, Lmax * 4], BF16, tag="al")
            al = al_fl[:].rearrange("p (s h) -> p s h", h=H)
            ali = nc.vector.tensor_tensor(out=al[:, 0:Lsb, :], in0=es_sl,
                                          in1=ed_sl, op=mybir.AluOpType.add)
            if li > 0:
                # keep the phase-E compacts ahead of phase-P DVE work: an
                # AG-gated al op scheduled between compacts head-of-line
                # blocks them, back-pressuring the ed gathers (et-tile WAR)
                add_dep_helper(ali.ins, compacts[soff].ins,
                               reason="phase P after compacts")
            # leaky-relu and exp in place (SBUF is tight)
            nc.vector.scalar_tensor_tensor(
                out=al[:, 0:Lsb, :], in0=al[:, 0:Lsb, :], scalar=NEG,
                op0=mybir.AluOpType.mult, in1=al[:, 0:Lsb, :],
                op1=mybir.AluOpType.max)
            ea = al
            nc.scalar.activation(out=ea[:, 0:Lsb, :], in_=al[:, 0:Lsb, :],
                                 func=mybir.ActivationFunctionType.Exp)

            rhs_fl = sb.tile([P, (Lmax + SB) * 260], BF16, tag="rhs")
            rhs = rhs_fl[:, 0:(Lsb + nb) * RC].rearrange(
                "p (s c) -> p s c", c=RC)
            if Lsb:
                nc.vector.tensor_tensor(
                    out=rhs[:, 0:Lsb, 0:OC].rearrange(
                        "p s (h o) -> p s h o", o=O),
                    in0=h_sl.rearrange("p s (h o) -> p s h o", o=O),
                    in1=ea[:, 0:Lsb, :].unsqueeze(3).to_broadcast(
                        [P, Lsb, H, O]),
                    op=mybir.AluOpType.mult)
                nc.vector.tensor_copy(out=rhs[:, 0:Lsb, OC:RC],
                                      in_=ea[:, 0:Lsb, :])

            m = sb.tile([P, Lmax * P], BF16, tag="m")
            mv = m[:].rearrange("p (s n) -> p s n", n=P)
            if Lsb:
                # one-hot build on DVE (walrus rejects is_equal on Pool)
                meng = nc.vector
                meng.tensor_tensor(
                    out=mv[:, 0:Lsb, :],
                    in0=c_dl[:, soff:soff + Lsb].unsqueeze(2).to_broadcast(
                        [P, Lsb, P]),
                    in1=c_iota[:].unsqueeze(1).to_broadcast([P, Lsb, P]),
                    op=mybir.AluOpType.is_equal)

            # self rows for this superblock's blocks, batched
            if li == 2:
                # L2 rows live in the packed SBUF tile already
                hbv = pkv[:, k0b:k1b, :]
            else:
                hbs = sb.tile([P, SB * USED[0]], BF16, tag="hbs")
                hbv = hbs[:, 0:nb * cols_u].rearrange("p (s c) -> p s c",
                                                      c=cols_u)
                sdeps = ([bar_local] if li == 0 else
                         [prev_hw_byblock[k] for k in range(k0b, k1b)
                          if k in prev_hw_byblock])
                nfb = min(k1b, nfull) - k0b
                if nfb > 0:
                    w = nc.sync.dma_start(
                        out=hbv[:, 0:nfb, :],
                        in_=self_src[k0b * P:(k0b + nfb) * P,
                                     0:cols_u].rearrange(
                                         "(a p) c -> p a c", p=P))
                    for dd in sdeps:
                        add_dep_helper(w.ins, dd.ins, reason="self rows dep")
                if k1b > nfull:
                    j = nfull - k0b
                    # verifier rejects partition-offset memsets; clear the
                    # whole block column, the partial load overwrites 0:rem
                    nc.vector.memset(hbv[:, j, :], 0.0)
                    w = nc.sync.dma_start(
                        out=hbv[:rem, j, :],
                        in_=self_src[nfull * P:NPD, 0:cols_u])
                    for dd in sdeps:
                        add_dep_helper(w.ins, dd.ins, reason="self rows dep")
            asl_s = sb.tile([P, SB * 4], BF16, tag="asls")
            aslv = asl_s[:, 0:nb * H].rearrange("p (s h) -> p s h", h=H)
            nc.vector.tensor_tensor(out=aslv[:], in0=hbv[:, :, es0:es0 + H],
                                    in1=hbv[:, :, es0 + H:es0 + 2 * H],
                                    op=mybir.AluOpType.add)
            nc.vector.scalar_tensor_tensor(
                out=aslv[:], in0=aslv[:], scalar=NEG,
                op0=mybir.AluOpType.mult, in1=aslv[:],
                op1=mybir.AluOpType.max)
            nc.scalar.activation(out=aslv[:], in_=aslv[:],
                                 func=mybir.ActivationFunctionType.Exp)
            # self slots for all nb blocks, batched
            nc.vector.tensor_tensor(
                out=rhs[:, Lsb:Lsb + nb, 0:OC].rearrange(
                    "p s (h o) -> p s h o", o=O),
                in0=hbv[:, :, 0:OC].rearrange("p s (h o) -> p s h o", o=O),
                in1=aslv[:].unsqueeze(3).to_broadcast([P, nb, H, O]),
                op=mybir.AluOpType.mult)
            nc.vector.tensor_copy(out=rhs[:, Lsb:Lsb + nb, OC:RC],
                                  in_=aslv[:])

            for b in range(nb):
                k = k0b + b
                nk = min(P, NPD - k * P)
                slo, shi = S_lo[k], S_hi[k]
                sidx = Lsb + b

                u_ps = ps_u.tile([P, RC], F32, space="PSUM", tag="u")
                ranges = []
                if slo:
                    p0 = blk_lo[k] - soff
                    ranges.append((p0, p0 + slo))
                if shi:
                    p0 = blk_hi[k] - soff
                    ranges.append((p0, p0 + shi))
                first = True
                for (r0, r1) in ranges:
                    for j in range(r0, r1):
                        nc.tensor.matmul(
                            out=u_ps[:], lhsT=m[:, j * P:(j + 1) * P],
                            rhs=rhs[:, j, :], start=first, stop=False)
                        first = False
                nc.tensor.matmul(out=u_ps[:], lhsT=c_identb[:],
                                 rhs=rhs[:, sidx, :], start=first, stop=True)

                rec = sb.tile([P, 4], F32, tag="rec")
                nc.vector.reciprocal(out=rec[:, 0:H], in_=u_ps[:, OC:RC])
                obb = sb.tile([P, 256], F32, tag="obb")
                nc.vector.tensor_tensor(
                    out=obb[:, 0:OC].rearrange("p (h o) -> p h o", o=O),
                    in0=u_ps[:, 0:OC].rearrange("p (h o) -> p h o", o=O),
                    in1=rec[:, 0:H].unsqueeze(2).to_broadcast([P, H, O]),
                    op=mybir.AluOpType.mult)
                if not plans["bzero"][li]:
                    nc.vector.tensor_tensor(out=obb[:, 0:OC],
                                            in0=obb[:, 0:OC],
                                            in1=c_b[li][:],
                                            op=mybir.AluOpType.add)

                if last:
                    if ostage is None:
                        ostage = sbs.tile([P, B_O, 2], F32, tag="ostage")
                        ostage_blocks = []
                    nc.scalar.activation(
                        out=ostage[:nk, k % B_O, :], in_=obb[:nk, 0:2],
                        func=mybir.ActivationFunctionType.Relu)
                    ostage_blocks.append((k % B_O, k, nk))
                    if len(ostage_blocks) == B_O or k == NBLK - 1:
                        full = [(j, kk) for (j, kk, nn) in ostage_blocks
                                if nn == P]
                        part = [(j, kk, nn) for (j, kk, nn) in ostage_blocks
                                if nn != P]
                        if full:
                            j0, k0 = full[0]
                            nc.sync.dma_start(
                                out=out_own[k0 * P:(k0 + len(full)) * P, :]
                                    .rearrange("(a p) c -> p a c", p=P),
                                in_=ostage[:, j0:j0 + len(full), :])
                        for (j, kk, nn) in part:
                            nc.sync.dma_start(
                                out=out_own[kk * P:kk * P + nn, :],
                                in_=ostage[:nn, j, :])
                        ostage = None
                else:
                    orl = sb.tile([P, 256], F32, tag="orl")
                    nc.scalar.activation(
                        out=orl[:], in_=obb[:, 0:OC],
                        func=mybir.ActivationFunctionType.Relu)
                    li2 = li + 1
                    cols2 = USED[li2]
                    h2_ps = ps.tile([P, max(cols2, 8)], F32, space="PSUM",
                                    tag="h2ps")
                    nf = OC // P
                    for f in range(nf):
                        tp_ps = ps.tile([P, P], F32, space="PSUM", tag="tp")
                        nc.tensor.transpose(
                            out=tp_ps[:], in_=orl[:, f * P:(f + 1) * P],
                            identity=c_identf[:])
                        xt = sb.tile([P, P], F32R, tag=f"xt{f}")
                        nc.scalar.copy(out=xt[:], in_=tp_ps[:])
                        nc.tensor.matmul(
                            out=h2_ps[:, 0:cols2], lhsT=xt[:],
                            rhs=wprime[li2][f][:, 0:cols2],
                            start=(f == 0), stop=(f == nf - 1))
                    wcols = cols2 if li2 == 1 else P
                    if hstage is None:
                        hstage = sbs.tile([P, B_H * USED[1]], BF16,
                                          tag="hstage")
                        hstage_blocks = []
                    hsv = hstage[:, 0:B_H * wcols].rearrange(
                        "p (s c) -> p s c", c=wcols)
                    nc.scalar.copy(out=hsv[:nk, k % B_H, 0:cols2],
                                   in_=h2_ps[:nk, 0:cols2])
                    hstage_blocks.append((k % B_H, k, nk))
                    if len(hstage_blocks) == B_H or k == NBLK - 1:
                        ws = flush_hown(hsv, hstage_blocks, li2)
                        for w in ws:
                            h_writes.append(w)
                            for (_, kk, _) in hstage_blocks:
                                hw_byblock[kk] = w
                        hstage = None


# --------------------------------------------------------------------------
# entry point
# --------------------------------------------------------------------------

_cache = {}
TRACE = False
last_result = None


def _plan_key(plans):
    return (plans["A"]["S_lo"], plans["A"]["S_hi"],
            plans["B"]["S_lo"], plans["B"]["S_hi"], plans["bzero"])


def kernel(x, edge_index, W0, a_src0, a_dst0, b0, W1, a_src1, a_dst1, b1,
           W2, a_src2, a_dst2, b2):
    weights = [(W0, a_src0, a_dst0, b0), (W1, a_src1, a_dst1, b1),
               (W2, a_src2, a_dst2, b2)]
    in_maps, plans = _host_prep(np.asarray(x), np.asarray(edge_index), weights)

    key = _plan_key(plans)
    if key not in _cache:
        _cache[key] = build_program(plans)
    nc = _cache[key]

    global last_result
    res = run_bass_kernel_spmd(nc, in_maps, core_ids=list(range(GEO.NCORES)),
                               trace=TRACE)
    last_result = res
    out = np.concatenate(
        [res.results[d]["out"] for d in range(GEO.NCORES)], axis=0)
    return out.astype(np.float32)

